# revision 15
# baseline (speedup 1.0000x reference)
"""4-layer GCN (N=100k, E=3.2M) on 8 TRN2 NeuronCores — aggregate-then-transform.

Key algebra: out = dis * ((A+I)-aggregate of dis*h) @ W + b, because the dense
transform commutes with the (linear) scatter-add aggregation. So we aggregate
the RAW pre-scaled features (fi-wide: 6/32/64/128) and apply W per-window
after aggregation; the [fi,ws]x[fi,fo] matmul also restores node-major
orientation for free. No per-node transform loops, no y tables, no PE
transposes anywhere.

Per layer: per dst-window -> indirect-DMA gather of hs[src] rows (fi wide),
one-hot H on DVE, scatter via PE matmul accumulation into zT [fi,P] PSUM,
W-matmul to o [ws,fo], epilogue dis-scale + bias (+relu, pre-scale for next
layer), write hs_next slice; AllGather hs_next across cores between layers.
Layer 4 ends with log_softmax on [ws,2].
"""

import sys

if "/opt/trn_rl_repo" not in sys.path:
    sys.path.insert(0, "/opt/trn_rl_repo")

import ml_dtypes
import numpy as np

import concourse.mybir as mybir
import concourse.tile as tile
from concourse import bacc
from concourse.bass import IndirectOffsetOnAxis
from concourse.bass_utils import run_bass_kernel_spmd

F32 = mybir.dt.float32
BF16 = mybir.dt.bfloat16
I32 = mybir.dt.int32
NPBF = ml_dtypes.bfloat16
P = 128
PAD_SLOT = 512.0

# ---------------------------------------------------------------- host side


def _assign_windows(deg, n_cores, ns):
    """Assign nodes to (core, window, slot), balancing per-window edge load."""
    n = deg.shape[0]
    nw = (ns + P - 1) // P
    last_cap = ns - (nw - 1) * P
    n_win = n_cores * nw
    caps = np.full(n_win, P, np.int64)
    caps[nw - 1 :: nw] = last_cap

    order = np.argsort(-deg, kind="stable")
    slots_of = [[] for _ in range(n_win)]
    win_ids = np.arange(n_win)
    pos = 0
    rnd = 0
    while pos < n:
        take = win_ids if rnd % 2 == 0 else win_ids[::-1]
        for wgid in take:
            if pos >= n:
                break
            if len(slots_of[wgid]) < caps[wgid]:
                slots_of[wgid].append(order[pos])
                pos += 1
        rnd += 1
    node_order = np.concatenate([np.array(s, np.int64) for s in slots_of])
    pos_of = np.empty(n, np.int64)
    pos_of[node_order] = np.arange(n)
    return node_order, pos_of, nw


def preprocess(x, edge_index, n_cores=8):
    n = x.shape[0]
    assert n % n_cores == 0
    ns = n // n_cores

    src = np.asarray(edge_index[0], np.int64)
    dst = np.asarray(edge_index[1], np.int64)

    # deg includes the self-loop (reference semantics); the self-loop edge
    # itself is NOT packed — it is applied on-device via an identity matmul.
    deg = np.bincount(dst, minlength=n) + 1
    node_order, pos_of, nw = _assign_windows(deg, n_cores, ns)

    dpos = pos_of[dst]
    spos = pos_of[src]
    e_core = dpos // ns
    off = dpos % ns
    e_w = off // P
    e_slot = off % P

    # sort edges by (core, window, src position) for gather locality
    key = (e_core * nw + e_w) * (n + 1) + spos
    eo = np.argsort(key, kind="stable")
    e_core = e_core[eo]
    e_w = e_w[eo]
    e_slot = e_slot[eo]
    spos = spos[eo]

    grp = e_core * nw + e_w
    cnt = np.bincount(grp, minlength=n_cores * nw).reshape(n_cores, nw)
    nb = int((cnt.max() + P - 1) // P)
    cnt_w = cnt.max(axis=0)              # max edges per window across cores
    tail_w = cnt_w - (np.maximum(cnt_w, 1) - 1) // P * P  # last-batch lanes

    start = np.zeros(n_cores * nw + 1, np.int64)
    start[1:] = np.cumsum(cnt.reshape(-1))
    e_k = np.arange(grp.shape[0]) - start[grp]
    e_lane = e_k % P
    e_batch = e_k // P

    src_pack = np.zeros((n_cores, P, nw * nb), np.int32)
    slot_pack = np.full((n_cores, P, nw * nb), PAD_SLOT, np.float32)
    col = e_w * nb + e_batch
    src_pack[e_core, e_lane, col] = spos.astype(np.int32)
    slot_pack[e_core, e_lane, col] = e_slot

    tail_w = np.maximum(tail_w, 2)       # single-lane indirect DMA unsupported

    deg_pack = np.ones((n_cores, P, nw), np.float32)
    gp = pos_of[node_order]
    deg_pack[gp // ns, (gp % ns) % P, (gp % ns) // P] = deg[node_order]

    x_sh = np.ascontiguousarray(
        x[node_order].reshape(n_cores, ns, x.shape[1])
    ).astype(np.float32)

    nb_w = ((cnt.max(axis=0) + P - 1) // P).astype(np.int64)
    cfg = dict(n=n, ns=ns, nw=nw, nb=nb, nb_w=nb_w, tail_w=tail_w,
               n_cores=n_cores)
    per_core = dict(src_pack=src_pack, slot_pack=slot_pack, deg_pack=deg_pack,
                    x_sh=x_sh)
    return cfg, per_core, node_order


# ---------------------------------------------------------------- device side


def build(cfg, dims, debug=False, reps=1, pipe=True, zero_bias=False,
          nb_cap=None):
    n, ns, nw, nb, n_cores = cfg["n"], cfg["ns"], cfg["nw"], cfg["nb"], cfg["n_cores"]
    nb_w = cfg.get("nb_w", np.full(nw, nb))
    tail_w = cfg.get("tail_w", np.full(nw, P))
    fin0 = dims[0][0]
    fo4 = dims[3][1]
    rg = [list(range(n_cores))]

    nc = bacc.Bacc(None, target_bir_lowering=False)

    # ---- I/O
    x_in = nc.dram_tensor("x_sh", [ns, fin0], F32, kind="ExternalInput")
    srcp_in = nc.dram_tensor("src_pack", [P, nw * nb], I32, kind="ExternalInput")
    slotp_in = nc.dram_tensor("slot_pack", [P, nw * nb], BF16,
                              kind="ExternalInput")
    deg_in = nc.dram_tensor("deg_pack", [P, nw], F32, kind="ExternalInput")
    w_in, bt_in = [], []
    for li, (fi, fo) in enumerate(dims):
        w_in.append(nc.dram_tensor(f"W{li + 1}", [fi, fo], BF16,
                                   kind="ExternalInput"))
        bt_in.append(nc.dram_tensor(f"bt{li + 1}", [P, fo], F32,
                                    kind="ExternalInput"))
    iota_in = nc.dram_tensor("iota", [P, P], BF16, kind="ExternalInput")
    ident_in = nc.dram_tensor("ident", [P, P], BF16, kind="ExternalInput")
    out_t = nc.dram_tensor("out", [ns, fo4], F32, kind="ExternalOutput")

    # ---- internal DRAM: per-layer gather tables (full, via AllGather)
    hs_loc = [nc.dram_tensor(f"hsl{li}", [ns, dims[li][0]], BF16)
              for li in range(4)]
    hs_full = [
        nc.dram_tensor(f"hsf{li}", [n, dims[li][0]], BF16, addr_space="Shared")
        for li in range(4)
    ]

    last_ws = ns - (nw - 1) * P

    with tile.TileContext(nc) as tc:
        with (
            tc.tile_pool(name="const", bufs=1) as cpool,
            tc.tile_pool(name="gpool", bufs=4) as gpool,
            tc.tile_pool(name="hpool", bufs=4) as hpool,
            tc.tile_pool(name="spool", bufs=4) as spool,
            tc.tile_pool(name="acc", bufs=4, space="PSUM") as accp,
            tc.tile_pool(name="acc2", bufs=4, space="PSUM") as accp2,
        ):
            # ---------- constants
            iota_t = cpool.tile([P, P], BF16, tag="iota")
            nc.sync.dma_start(iota_t[:], iota_in[:, :])
            ident_t = cpool.tile([P, P], BF16, tag="ident")
            nc.sync.dma_start(ident_t[:], ident_in[:, :])
            w_t, bt_t = [], []
            for li, (fi, fo) in enumerate(dims):
                wt = cpool.tile([fi, fo], BF16, tag=f"w{li}")
                nc.sync.dma_start(wt[:], w_in[li][:, :])
                w_t.append(wt)
                bt = cpool.tile([P, fo], F32, tag=f"bt{li}")
                nc.sync.dma_start(bt[:], bt_in[li][:, :])
                bt_t.append(bt)
            src_t = cpool.tile([P, nw * nb], I32, tag="srcp")
            nc.sync.dma_start(src_t[:], srcp_in[:, :])
            slot_t = cpool.tile([P, nw * nb], BF16, tag="slotp")
            nc.sync.dma_start(slot_t[:], slotp_in[:, :])

            # ---------- dis = 1/sqrt(deg)
            deg_t = cpool.tile([P, nw], F32, tag="deg")
            nc.sync.dma_start(deg_t[:], deg_in[:, :])
            rec_t = cpool.tile([P, nw], F32, tag="rec")
            nc.vector.reciprocal(out=rec_t[:], in_=deg_t[:])
            dis_t = cpool.tile([P, nw], F32, tag="dis")
            nc.scalar.sqrt(out=dis_t[:], in_=rec_t[:])
            dis2_t = cpool.tile([P, nw], F32, tag="dis2")
            nc.vector.tensor_tensor(
                out=dis2_t[:], in0=dis_t[:], in1=dis_t[:],
                op=mybir.AluOpType.mult,
            )

            # ---------- phase 0: hs0 = dis * x  (bf16, node-major)
            # g pool buffers are read at stale lanes by partial-tail batches
            # (killed by zero one-hot columns); memset once so stale != NaN.
            for _ in range(4):
                gz = gpool.tile([P, nb * 128], BF16, tag="g")
                nc.vector.memset(gz[:], 0.0)
            hs_sb = {}
            for w in range(nw):
                ws = P if w < nw - 1 else last_ws
                t0 = w * P
                xt = spool.tile([P, fin0], F32, tag="xt")
                nc.sync.dma_start(xt[:ws], x_in[t0 : t0 + ws, :])
                xs = cpool.tile([P, fin0], BF16, tag=f"hs0_{w}")
                nc.scalar.activation(
                    out=xs[:ws],
                    in_=xt[:ws],
                    func=mybir.ActivationFunctionType.Copy,
                    scale=dis_t[:ws, w : w + 1],
                )
                hs_sb[(0, w)] = xs
                nc.sync.dma_start(hs_loc[0][t0 : t0 + ws, :], xs[:ws])
            nc.gpsimd.collective_compute(
                "AllGather",
                mybir.AluOpType.bypass,
                replica_groups=rg,
                ins=[hs_loc[0].ap().opt()],
                outs=[hs_full[0].ap().opt()],
            )

            # ---------- layers
            for rep in range(reps):
              for li in range(4):
                  fi, fo = dims[li]

                  def epi(zT_sb, ws, t0, w, li=li, fi=fi, fo=fo):
                      o = accp2.tile([P, fo], F32, tag="o")
                      nc.tensor.matmul(
                          out=o[:ws],
                          lhsT=zT_sb[:, :ws],
                          rhs=w_t[li][:],
                          start=True,
                          stop=True,
                      )
                      if zero_bias and li < 3:
                          hsn = cpool.tile([P, fo], BF16, tag=f"hs{li + 1}_{w}")
                          nc.scalar.activation(
                              out=hsn[:ws],
                              in_=o[:ws],
                              func=mybir.ActivationFunctionType.Relu,
                              scale=dis2_t[:ws, w : w + 1],
                          )
                          hs_sb[(li + 1, w)] = hsn
                          nc.sync.dma_start(
                              hs_loc[li + 1][t0 : t0 + ws, :], hsn[:ws]
                          )
                          return
                      if zero_bias:
                          v = spool.tile([P, fo], F32, tag="v")
                          nc.scalar.activation(
                              out=v[:ws],
                              in_=o[:ws],
                              func=mybir.ActivationFunctionType.Copy,
                              scale=dis_t[:ws, w : w + 1],
                          )
                      else:
                          u = spool.tile([P, fo], F32, tag="u")
                          nc.scalar.activation(
                              out=u[:ws],
                              in_=o[:ws],
                              func=mybir.ActivationFunctionType.Copy,
                              scale=dis_t[:ws, w : w + 1],
                          )
                          v = spool.tile([P, fo], F32, tag="v")
                          nc.vector.tensor_tensor(
                              out=v[:ws], in0=u[:ws], in1=bt_t[li][:ws],
                              op=mybir.AluOpType.add,
                          )
                      if li < 3:
                          hsn = cpool.tile([P, fo], BF16, tag=f"hs{li + 1}_{w}")
                          nc.scalar.activation(
                              out=hsn[:ws],
                              in_=v[:ws],
                              func=mybir.ActivationFunctionType.Relu,
                              scale=dis_t[:ws, w : w + 1],
                          )
                          hs_sb[(li + 1, w)] = hsn
                          nc.sync.dma_start(
                              hs_loc[li + 1][t0 : t0 + ws, :], hsn[:ws]
                          )
                      else:
                          nm = spool.tile([P, 1], F32, tag="nm")
                          nc.vector.tensor_reduce(
                              out=nm[:ws],
                              in_=v[:ws],
                              op=mybir.AluOpType.max,
                              axis=mybir.AxisListType.X,
                              negate=True,
                          )
                          e = spool.tile([P, fo], F32, tag="e")
                          nc.scalar.activation(
                              out=e[:ws],
                              in_=v[:ws],
                              func=mybir.ActivationFunctionType.Exp,
                              bias=nm[:ws],
                          )
                          s = spool.tile([P, 1], F32, tag="s")
                          nc.vector.tensor_reduce(
                              out=s[:ws],
                              in_=e[:ws],
                              op=mybir.AluOpType.add,
                              axis=mybir.AxisListType.X,
                          )
                          ls = spool.tile([P, 1], F32, tag="ls")
                          nc.scalar.activation(
                              out=ls[:ws], in_=s[:ws],
                              func=mybir.ActivationFunctionType.Ln,
                          )
                          r_t = spool.tile([P, fo], F32, tag="r")
                          nc.vector.tensor_scalar(
                              out=r_t[:ws],
                              in0=v[:ws],
                              scalar1=nm[:ws],
                              scalar2=ls[:ws],
                              op0=mybir.AluOpType.add,
                              op1=mybir.AluOpType.subtract,
                          )
                          nc.sync.dma_start(out_t[t0 : t0 + ws, :], r_t[:ws])

                  pend = None
                  for w in range(nw):
                      ws = P if w < nw - 1 else last_ws
                      t0 = w * P
                      nbw = int(nb_w[w]) if nb_cap is None else min(
                          int(nb_w[w]), nb_cap)
                      tail = int(tail_w[w]) if nb_cap is None else P
                      g = gpool.tile([P, nb * fi], BF16, tag="g")
                      if nbw > 1:
                          nc.gpsimd.indirect_dma_start(
                              out=g[:, : (nbw - 1) * fi],
                              out_offset=None,
                              in_=hs_full[li][:, :],
                              in_offset=IndirectOffsetOnAxis(
                                  ap=src_t[:, w * nb : w * nb + nbw - 1],
                                  axis=0,
                              ),
                          )
                      nc.gpsimd.indirect_dma_start(
                          out=g[:tail, (nbw - 1) * fi : nbw * fi],
                          out_offset=None,
                          in_=hs_full[li][:, :],
                          in_offset=IndirectOffsetOnAxis(
                              ap=src_t[:tail, w * nb + nbw - 1 : w * nb + nbw],
                              axis=0,
                          ),
                      )
                      h = hpool.tile([P, nb * P], BF16, tag="h")
                      nc.vector.tensor_tensor(
                          out=h[:, : nbw * P].rearrange(
                              "p (b s) -> p b s", b=nbw),
                          in0=slot_t[:, w * nb : w * nb + nbw].to_broadcast(
                              [P, nbw, P]
                          ),
                          in1=iota_t[:]
                          .rearrange("p (b s) -> p b s", b=1)
                          .to_broadcast([P, nbw, P]),
                          op=mybir.AluOpType.is_equal,
                      )
                      zT = accp.tile([fi, P], F32, tag="acc")
                      # self-loop contribution: zT[:, s] += hs[s] for the
                      # window's own nodes (kept resident in SBUF)
                      nc.tensor.matmul(
                          out=zT[:],
                          lhsT=hs_sb[(li, w)][:ws],
                          rhs=ident_t[:ws],
                          start=True,
                          stop=False,
                      )
                      for b in range(nbw):
                          nc.tensor.matmul(
                              out=zT[:],
                              lhsT=g[:, b * fi : (b + 1) * fi],
                              rhs=h[:, b * P : (b + 1) * P],
                              start=False,
                              stop=(b == nbw - 1),
                          )
                      zT_sb = spool.tile([fi, P], BF16, tag="zTsb")
                      nc.vector.tensor_copy(out=zT_sb[:], in_=zT[:])
                      if pipe:
                          if pend is not None:
                              epi(*pend)
                          pend = (zT_sb, ws, t0, w)
                      else:
                          epi(zT_sb, ws, t0, w)
                  if pend is not None:
                      epi(*pend)

                  if li < 3:
                      nc.gpsimd.collective_compute(
                          "AllGather",
                          mybir.AluOpType.bypass,
                          replica_groups=rg,
                          ins=[hs_loc[li + 1].ap().opt()],
                          outs=[hs_full[li + 1].ap().opt()],
                      )

    nc.finalize()
    return nc


def make_in_maps(cfg, per_core, Ws, bs):
    common = dict(iota=np.tile(np.arange(P, dtype=NPBF), (P, 1)),
                  ident=np.eye(P, dtype=NPBF))
    for li in range(4):
        common[f"W{li + 1}"] = np.asarray(Ws[li], np.float32).astype(NPBF)
        common[f"bt{li + 1}"] = np.tile(np.asarray(bs[li], np.float32), (P, 1))
    in_maps = []
    for c in range(cfg["n_cores"]):
        m = dict(common)
        m["x_sh"] = per_core["x_sh"][c]
        m["src_pack"] = per_core["src_pack"][c]
        m["slot_pack"] = per_core["slot_pack"][c].astype(NPBF)
        m["deg_pack"] = per_core["deg_pack"][c]
        in_maps.append(m)
    return in_maps


# ---------------------------------------------------------------- entry point


def kernel(x, edge_index, W1, b1, W2, b2, W3, b3, W4, b4):
    x = np.asarray(x, np.float32)
    n_cores = 8
    cfg, per_core, node_order = preprocess(x, np.asarray(edge_index), n_cores)
    dims = [
        (W1.shape[0], W1.shape[1]),
        (W2.shape[0], W2.shape[1]),
        (W3.shape[0], W3.shape[1]),
        (W4.shape[0], W4.shape[1]),
    ]
    zb = all(not np.any(np.asarray(b)) for b in (b1, b2, b3, b4))
    nc = build(cfg, dims, zero_bias=zb)
    in_maps = make_in_maps(cfg, per_core, (W1, W2, W3, W4), (b1, b2, b3, b4))
    res = run_bass_kernel_spmd(nc, in_maps, list(range(n_cores)))
    outs = np.concatenate([res.results[c]["out"] for c in range(n_cores)], axis=0)
    full = np.empty((cfg["n"], dims[3][1]), np.float32)
    full[node_order] = outs
    return full



# revision 24
# speedup vs baseline: 1.0227x; 1.0227x over previous
"""4-layer GCN (N=100k, E=3.2M) on 8 TRN2 NeuronCores — aggregate-then-transform.

Key algebra: out = dis * ((A+I)-aggregate of dis*h) @ W + b, because the dense
transform commutes with the (linear) scatter-add aggregation. So we aggregate
the RAW pre-scaled features (fi-wide: 6/32/64/128) and apply W per-window
after aggregation; the [fi,ws]x[fi,fo] matmul also restores node-major
orientation for free. No per-node transform loops, no y tables, no PE
transposes anywhere.

Per layer: per dst-window -> indirect-DMA gather of hs[src] rows (fi wide),
one-hot H on DVE, scatter via PE matmul accumulation into zT [fi,P] PSUM,
W-matmul to o [ws,fo], epilogue dis-scale + bias (+relu, pre-scale for next
layer), write hs_next slice; AllGather hs_next across cores between layers.
Layer 4 ends with log_softmax on [ws,2].
"""

import sys

if "/opt/trn_rl_repo" not in sys.path:
    sys.path.insert(0, "/opt/trn_rl_repo")

import ml_dtypes
import numpy as np

import concourse.mybir as mybir
import concourse.tile as tile
from concourse import bacc
from concourse.bass import IndirectOffsetOnAxis
from concourse.bass_utils import run_bass_kernel_spmd

F32 = mybir.dt.float32
BF16 = mybir.dt.bfloat16
I32 = mybir.dt.int32
NPBF = ml_dtypes.bfloat16
P = 128
PAD_SLOT = 512.0

# ---------------------------------------------------------------- host side


def _assign_windows(deg, n_cores, ns):
    """Assign nodes to (core, window, slot), balancing per-window edge load."""
    n = deg.shape[0]
    nw = (ns + P - 1) // P
    last_cap = ns - (nw - 1) * P
    n_win = n_cores * nw
    caps = np.full(n_win, P, np.int64)
    caps[nw - 1 :: nw] = last_cap

    order = np.argsort(-deg, kind="stable")
    slots_of = [[] for _ in range(n_win)]
    win_ids = np.arange(n_win)
    pos = 0
    rnd = 0
    while pos < n:
        take = win_ids if rnd % 2 == 0 else win_ids[::-1]
        for wgid in take:
            if pos >= n:
                break
            if len(slots_of[wgid]) < caps[wgid]:
                slots_of[wgid].append(order[pos])
                pos += 1
        rnd += 1
    node_order = np.concatenate([np.array(s, np.int64) for s in slots_of])
    pos_of = np.empty(n, np.int64)
    pos_of[node_order] = np.arange(n)
    return node_order, pos_of, nw


def preprocess(x, edge_index, n_cores=8):
    n = x.shape[0]
    assert n % n_cores == 0
    ns = n // n_cores

    src = np.asarray(edge_index[0], np.int64)
    dst = np.asarray(edge_index[1], np.int64)

    # deg includes the self-loop (reference semantics); the self-loop edge
    # itself is NOT packed — it is applied on-device via an identity matmul.
    deg = np.bincount(dst, minlength=n) + 1
    node_order, pos_of, nw = _assign_windows(deg, n_cores, ns)

    dpos = pos_of[dst]
    spos = pos_of[src]
    e_core = dpos // ns
    off = dpos % ns
    e_w = off // P
    e_slot = off % P

    # sort edges by (core, window, src position) for gather locality
    key = (e_core * nw + e_w) * (n + 1) + spos
    eo = np.argsort(key, kind="stable")
    e_core = e_core[eo]
    e_w = e_w[eo]
    e_slot = e_slot[eo]
    spos = spos[eo]

    grp = e_core * nw + e_w
    cnt = np.bincount(grp, minlength=n_cores * nw).reshape(n_cores, nw)
    nb = int((cnt.max() + P - 1) // P)
    cnt_w = cnt.max(axis=0)              # max edges per window across cores
    tail_w = cnt_w - (np.maximum(cnt_w, 1) - 1) // P * P  # last-batch lanes

    start = np.zeros(n_cores * nw + 1, np.int64)
    start[1:] = np.cumsum(cnt.reshape(-1))
    e_k = np.arange(grp.shape[0]) - start[grp]
    e_lane = e_k % P
    e_batch = e_k // P

    src_pack = np.zeros((n_cores, P, nw * nb), np.int32)
    slot_pack = np.full((n_cores, P, nw * nb), PAD_SLOT, np.float32)
    col = e_w * nb + e_batch
    src_pack[e_core, e_lane, col] = spos.astype(np.int32)
    slot_pack[e_core, e_lane, col] = e_slot

    tail_w = np.maximum(tail_w, 2)       # single-lane indirect DMA unsupported

    deg_pack = np.ones((n_cores, P, nw), np.float32)
    gp = pos_of[node_order]
    deg_pack[gp // ns, (gp % ns) % P, (gp % ns) // P] = deg[node_order]

    x_sh = np.ascontiguousarray(
        x[node_order].reshape(n_cores, ns, x.shape[1])
    ).astype(np.float32)

    nb_w = ((cnt.max(axis=0) + P - 1) // P).astype(np.int64)
    cfg = dict(n=n, ns=ns, nw=nw, nb=nb, nb_w=nb_w, tail_w=tail_w,
               n_cores=n_cores)
    per_core = dict(src_pack=src_pack, slot_pack=slot_pack, deg_pack=deg_pack,
                    x_sh=x_sh)
    return cfg, per_core, node_order


# ---------------------------------------------------------------- device side


def build(cfg, dims, debug=False, reps=1, pipe=True, zero_bias=False,
          nb_cap=None):
    n, ns, nw, nb, n_cores = cfg["n"], cfg["ns"], cfg["nw"], cfg["nb"], cfg["n_cores"]
    nb_w = cfg.get("nb_w", np.full(nw, nb))
    tail_w = cfg.get("tail_w", np.full(nw, P))
    fin0 = dims[0][0]
    fo4 = dims[3][1]
    rg = [list(range(n_cores))]

    nc = bacc.Bacc(None, target_bir_lowering=False)

    # ---- I/O
    x_in = nc.dram_tensor("x_sh", [ns, fin0], F32, kind="ExternalInput")
    srcp_in = nc.dram_tensor("src_pack", [P, nw * nb], I32, kind="ExternalInput")
    slotp_in = nc.dram_tensor("slot_pack", [P, nw * nb], BF16,
                              kind="ExternalInput")
    deg_in = nc.dram_tensor("deg_pack", [P, nw], F32, kind="ExternalInput")
    w_in, bt_in = [], []
    for li, (fi, fo) in enumerate(dims):
        w_in.append(nc.dram_tensor(f"W{li + 1}", [fi, fo], BF16,
                                   kind="ExternalInput"))
        bt_in.append(nc.dram_tensor(f"bt{li + 1}", [P, fo], F32,
                                    kind="ExternalInput"))
    iota_in = nc.dram_tensor("iota", [P, P], BF16, kind="ExternalInput")
    ident_in = nc.dram_tensor("ident", [P, P], BF16, kind="ExternalInput")
    out_t = nc.dram_tensor("out", [ns, fo4], F32, kind="ExternalOutput")

    # ---- internal DRAM: per-layer gather tables (full, via AllGather)
    hs_loc = [nc.dram_tensor(f"hsl{li}", [ns, dims[li][0]], BF16)
              for li in range(4)]
    hs_full = [
        nc.dram_tensor(f"hsf{li}", [n, dims[li][0]], BF16, addr_space="Shared")
        for li in range(4)
    ]

    last_ws = ns - (nw - 1) * P

    with tile.TileContext(nc) as tc:
        with (
            tc.tile_pool(name="const", bufs=1) as cpool,
            tc.tile_pool(name="gpool", bufs=4) as gpool,
            tc.tile_pool(name="hpool", bufs=4) as hpool,
            tc.tile_pool(name="spool", bufs=4) as spool,
            tc.tile_pool(name="acc", bufs=4, space="PSUM") as accp,
            tc.tile_pool(name="acc2", bufs=4, space="PSUM") as accp2,
        ):
            # ---------- constants
            iota_t = cpool.tile([P, P], BF16, tag="iota")
            nc.sync.dma_start(iota_t[:], iota_in[:, :])
            ident_t = cpool.tile([P, P], BF16, tag="ident")
            nc.sync.dma_start(ident_t[:], ident_in[:, :])
            w_t, bt_t = [], []
            for li, (fi, fo) in enumerate(dims):
                wt = cpool.tile([fi, fo], BF16, tag=f"w{li}")
                nc.sync.dma_start(wt[:], w_in[li][:, :])
                w_t.append(wt)
                bt = cpool.tile([P, fo], F32, tag=f"bt{li}")
                nc.sync.dma_start(bt[:], bt_in[li][:, :])
                bt_t.append(bt)
            src_t = cpool.tile([P, nw * nb], I32, tag="srcp")
            nc.sync.dma_start(src_t[:], srcp_in[:, :])
            slot_t = cpool.tile([P, nw * nb], BF16, tag="slotp")
            nc.sync.dma_start(slot_t[:], slotp_in[:, :])

            # ---------- dis = 1/sqrt(deg)
            deg_t = cpool.tile([P, nw], F32, tag="deg")
            nc.sync.dma_start(deg_t[:], deg_in[:, :])
            rec_t = cpool.tile([P, nw], F32, tag="rec")
            nc.vector.reciprocal(out=rec_t[:], in_=deg_t[:])
            dis_t = cpool.tile([P, nw], F32, tag="dis")
            nc.scalar.sqrt(out=dis_t[:], in_=rec_t[:])
            dis2_t = cpool.tile([P, nw], F32, tag="dis2")
            nc.vector.tensor_tensor(
                out=dis2_t[:], in0=dis_t[:], in1=dis_t[:],
                op=mybir.AluOpType.mult,
            )

            # ---------- persistent per-layer feature tiles (node w*128+p at
            # [p, w*fi : (w+1)*fi]); layer li's windows read their own rows
            # from hs_lay[li] for the self-loop identity matmul.
            nwf = nw - 1                     # count of full 128-node windows
            hs_lay = [
                cpool.tile([P, nw * dims[li][0]], BF16, tag=f"hslay{li}",
                           name=f"hslay{li}")
                for li in range(4)
            ]
            out_lay = cpool.tile([P, nw * fo4], F32, tag="outlay")

            def store_hs(lay, fo, loc):
                """hs_lay -> node-major DRAM table in 2 DMAs."""
                nc.sync.dma_start(
                    loc[: nwf * P, :].rearrange("(w p) f -> p w f", p=P),
                    lay[:, : nwf * fo].rearrange("p (w f) -> p w f", w=nwf),
                )
                nc.sync.dma_start(
                    loc[nwf * P :, :], lay[:last_ws, nwf * fo :]
                )

            # ---------- phase 0: hs0 = dis * x  (bf16)
            xstage = cpool.tile([P, nw * fin0], F32, tag="xstage")
            nc.sync.dma_start(
                xstage[:, : nwf * fin0].rearrange("p (w f) -> p w f", w=nwf),
                x_in[: nwf * P, :].rearrange("(w p) f -> p w f", p=P),
            )
            nc.sync.dma_start(
                xstage[:last_ws, nwf * fin0 :], x_in[nwf * P :, :]
            )
            for w in range(nw):
                ws = P if w < nw - 1 else last_ws
                nc.scalar.activation(
                    out=hs_lay[0][:ws, w * fin0 : (w + 1) * fin0],
                    in_=xstage[:ws, w * fin0 : (w + 1) * fin0],
                    func=mybir.ActivationFunctionType.Copy,
                    scale=dis_t[:ws, w : w + 1],
                )
            store_hs(hs_lay[0], fin0, hs_loc[0])
            nc.gpsimd.collective_compute(
                "AllGather",
                mybir.AluOpType.bypass,
                replica_groups=rg,
                ins=[hs_loc[0].ap().opt()],
                outs=[hs_full[0].ap().opt()],
            )

            # ---------- layers
            for rep in range(reps):
              for li in range(4):
                  fi, fo = dims[li]

                  def epi(zT_sb, ws, t0, w, li=li, fi=fi, fo=fo):
                      o = accp2.tile([P, fo], F32, tag="o")
                      nc.tensor.matmul(
                          out=o[:ws],
                          lhsT=zT_sb[:, :ws],
                          rhs=w_t[li][:],
                          start=True,
                          stop=True,
                      )
                      if zero_bias and li < 3:
                          nc.scalar.activation(
                              out=hs_lay[li + 1][:ws, w * fo : (w + 1) * fo],
                              in_=o[:ws],
                              func=mybir.ActivationFunctionType.Relu,
                              scale=dis2_t[:ws, w : w + 1],
                          )
                          return
                      if zero_bias:
                          v = spool.tile([P, fo], F32, tag="v")
                          nc.scalar.activation(
                              out=v[:ws],
                              in_=o[:ws],
                              func=mybir.ActivationFunctionType.Copy,
                              scale=dis_t[:ws, w : w + 1],
                          )
                      else:
                          u = spool.tile([P, fo], F32, tag="u")
                          nc.scalar.activation(
                              out=u[:ws],
                              in_=o[:ws],
                              func=mybir.ActivationFunctionType.Copy,
                              scale=dis_t[:ws, w : w + 1],
                          )
                          v = spool.tile([P, fo], F32, tag="v")
                          nc.vector.tensor_tensor(
                              out=v[:ws], in0=u[:ws], in1=bt_t[li][:ws],
                              op=mybir.AluOpType.add,
                          )
                      if li < 3:
                          nc.scalar.activation(
                              out=hs_lay[li + 1][:ws, w * fo : (w + 1) * fo],
                              in_=v[:ws],
                              func=mybir.ActivationFunctionType.Relu,
                              scale=dis_t[:ws, w : w + 1],
                          )
                      else:
                          nm = spool.tile([P, 1], F32, tag="nm")
                          nc.vector.tensor_reduce(
                              out=nm[:ws],
                              in_=v[:ws],
                              op=mybir.AluOpType.max,
                              axis=mybir.AxisListType.X,
                              negate=True,
                          )
                          e = spool.tile([P, fo], F32, tag="e")
                          nc.scalar.activation(
                              out=e[:ws],
                              in_=v[:ws],
                              func=mybir.ActivationFunctionType.Exp,
                              bias=nm[:ws],
                          )
                          s = spool.tile([P, 1], F32, tag="s")
                          nc.vector.tensor_reduce(
                              out=s[:ws],
                              in_=e[:ws],
                              op=mybir.AluOpType.add,
                              axis=mybir.AxisListType.X,
                          )
                          ls = spool.tile([P, 1], F32, tag="ls")
                          nc.scalar.activation(
                              out=ls[:ws], in_=s[:ws],
                              func=mybir.ActivationFunctionType.Ln,
                          )
                          nc.vector.tensor_scalar(
                              out=out_lay[:ws, w * fo : (w + 1) * fo],
                              in0=v[:ws],
                              scalar1=nm[:ws],
                              scalar2=ls[:ws],
                              op0=mybir.AluOpType.add,
                              op1=mybir.AluOpType.subtract,
                          )

                  pend = None
                  for w in range(nw):
                      ws = P if w < nw - 1 else last_ws
                      t0 = w * P
                      nbw = int(nb_w[w]) if nb_cap is None else min(
                          int(nb_w[w]), nb_cap)
                      g = gpool.tile([P, nb * fi], BF16, tag="g")
                      nc.gpsimd.indirect_dma_start(
                          out=g[:, : nbw * fi],
                          out_offset=None,
                          in_=hs_full[li][:, :],
                          in_offset=IndirectOffsetOnAxis(
                              ap=src_t[:, w * nb : w * nb + nbw], axis=0
                          ),
                      )
                      h = hpool.tile([P, nb * P], BF16, tag="h")
                      nc.vector.tensor_tensor(
                          out=h[:, : nbw * P].rearrange(
                              "p (b s) -> p b s", b=nbw),
                          in0=slot_t[:, w * nb : w * nb + nbw].to_broadcast(
                              [P, nbw, P]
                          ),
                          in1=iota_t[:]
                          .rearrange("p (b s) -> p b s", b=1)
                          .to_broadcast([P, nbw, P]),
                          op=mybir.AluOpType.is_equal,
                      )
                      zT = accp.tile([fi, P], F32, tag="acc")
                      # self-loop contribution: zT[:, s] += hs[s] for the
                      # window's own nodes (kept resident in SBUF)
                      nc.tensor.matmul(
                          out=zT[:],
                          lhsT=hs_lay[li][:ws, w * fi : (w + 1) * fi],
                          rhs=ident_t[:ws],
                          start=True,
                          stop=False,
                      )
                      for b in range(nbw):
                          nc.tensor.matmul(
                              out=zT[:],
                              lhsT=g[:, b * fi : (b + 1) * fi],
                              rhs=h[:, b * P : (b + 1) * P],
                              start=False,
                              stop=(b == nbw - 1),
                          )
                      zT_sb = spool.tile([fi, P], BF16, tag="zTsb")
                      nc.vector.tensor_copy(out=zT_sb[:], in_=zT[:])
                      if pipe:
                          if pend is not None:
                              epi(*pend)
                          pend = (zT_sb, ws, t0, w)
                      else:
                          epi(zT_sb, ws, t0, w)
                  if pend is not None:
                      epi(*pend)

                  if li < 3:
                      store_hs(hs_lay[li + 1], dims[li + 1][0],
                               hs_loc[li + 1])
                      nc.gpsimd.collective_compute(
                          "AllGather",
                          mybir.AluOpType.bypass,
                          replica_groups=rg,
                          ins=[hs_loc[li + 1].ap().opt()],
                          outs=[hs_full[li + 1].ap().opt()],
                      )
                  else:
                      nc.sync.dma_start(
                          out_t[: nwf * P, :].rearrange(
                              "(w p) f -> p w f", p=P),
                          out_lay[:, : nwf * fo4].rearrange(
                              "p (w f) -> p w f", w=nwf),
                      )
                      nc.sync.dma_start(
                          out_t[nwf * P :, :], out_lay[:last_ws, nwf * fo4 :]
                      )

    nc.finalize()
    return nc


def make_in_maps(cfg, per_core, Ws, bs):
    common = dict(iota=np.tile(np.arange(P, dtype=NPBF), (P, 1)),
                  ident=np.eye(P, dtype=NPBF))
    for li in range(4):
        common[f"W{li + 1}"] = np.asarray(Ws[li], np.float32).astype(NPBF)
        common[f"bt{li + 1}"] = np.tile(np.asarray(bs[li], np.float32), (P, 1))
    in_maps = []
    for c in range(cfg["n_cores"]):
        m = dict(common)
        m["x_sh"] = per_core["x_sh"][c]
        m["src_pack"] = per_core["src_pack"][c]
        m["slot_pack"] = per_core["slot_pack"][c].astype(NPBF)
        m["deg_pack"] = per_core["deg_pack"][c]
        in_maps.append(m)
    return in_maps


# ---------------------------------------------------------------- entry point


def kernel(x, edge_index, W1, b1, W2, b2, W3, b3, W4, b4):
    x = np.asarray(x, np.float32)
    n_cores = 8
    cfg, per_core, node_order = preprocess(x, np.asarray(edge_index), n_cores)
    dims = [
        (W1.shape[0], W1.shape[1]),
        (W2.shape[0], W2.shape[1]),
        (W3.shape[0], W3.shape[1]),
        (W4.shape[0], W4.shape[1]),
    ]
    zb = all(not np.any(np.asarray(b)) for b in (b1, b2, b3, b4))
    nc = build(cfg, dims, zero_bias=zb)
    in_maps = make_in_maps(cfg, per_core, (W1, W2, W3, W4), (b1, b2, b3, b4))
    res = run_bass_kernel_spmd(nc, in_maps, list(range(n_cores)))
    outs = np.concatenate([res.results[c]["out"] for c in range(n_cores)], axis=0)
    full = np.empty((cfg["n"], dims[3][1]), np.float32)
    full[node_order] = outs
    return full



# revision 25
# speedup vs baseline: 1.0457x; 1.0224x over previous
"""4-layer GCN (N=100k, E=3.2M) on 8 TRN2 NeuronCores — aggregate-then-transform.

Key algebra: out = dis * ((A+I)-aggregate of dis*h) @ W + b, because the dense
transform commutes with the (linear) scatter-add aggregation. So we aggregate
the RAW pre-scaled features (fi-wide: 6/32/64/128) and apply W per-window
after aggregation; the [fi,ws]x[fi,fo] matmul also restores node-major
orientation for free. No per-node transform loops, no y tables, no PE
transposes anywhere.

Per layer: per dst-window -> indirect-DMA gather of hs[src] rows (fi wide),
one-hot H on DVE, scatter via PE matmul accumulation into zT [fi,P] PSUM,
W-matmul to o [ws,fo], epilogue dis-scale + bias (+relu, pre-scale for next
layer), write hs_next slice; AllGather hs_next across cores between layers.
Layer 4 ends with log_softmax on [ws,2].
"""

import sys

if "/opt/trn_rl_repo" not in sys.path:
    sys.path.insert(0, "/opt/trn_rl_repo")

import ml_dtypes
import numpy as np

import concourse.mybir as mybir
import concourse.tile as tile
from concourse import bacc
from concourse.bass import IndirectOffsetOnAxis
from concourse.bass_utils import run_bass_kernel_spmd

F32 = mybir.dt.float32
BF16 = mybir.dt.bfloat16
I32 = mybir.dt.int32
NPBF = ml_dtypes.bfloat16
P = 128
PAD_SLOT = 512.0

# ---------------------------------------------------------------- host side


def _assign_windows(deg, n_cores, ns):
    """Assign nodes to (core, window, slot), balancing per-window edge load."""
    n = deg.shape[0]
    nw = (ns + P - 1) // P
    last_cap = ns - (nw - 1) * P
    n_win = n_cores * nw
    caps = np.full(n_win, P, np.int64)
    caps[nw - 1 :: nw] = last_cap

    order = np.argsort(-deg, kind="stable")
    slots_of = [[] for _ in range(n_win)]
    win_ids = np.arange(n_win)
    pos = 0
    rnd = 0
    while pos < n:
        take = win_ids if rnd % 2 == 0 else win_ids[::-1]
        for wgid in take:
            if pos >= n:
                break
            if len(slots_of[wgid]) < caps[wgid]:
                slots_of[wgid].append(order[pos])
                pos += 1
        rnd += 1
    node_order = np.concatenate([np.array(s, np.int64) for s in slots_of])
    pos_of = np.empty(n, np.int64)
    pos_of[node_order] = np.arange(n)
    return node_order, pos_of, nw


def preprocess(x, edge_index, n_cores=8):
    n = x.shape[0]
    assert n % n_cores == 0
    ns = n // n_cores

    src = np.asarray(edge_index[0], np.int64)
    dst = np.asarray(edge_index[1], np.int64)

    # deg includes the self-loop (reference semantics); the self-loop edge
    # itself is NOT packed — it is applied on-device via an identity matmul.
    deg = np.bincount(dst, minlength=n) + 1
    node_order, pos_of, nw = _assign_windows(deg, n_cores, ns)

    dpos = pos_of[dst]
    spos = pos_of[src]
    e_core = dpos // ns
    off = dpos % ns
    e_w = off // P
    e_slot = off % P

    # sort edges by (core, window, src position) for gather locality
    key = (e_core * nw + e_w) * (n + 1) + spos
    eo = np.argsort(key, kind="stable")
    e_core = e_core[eo]
    e_w = e_w[eo]
    e_slot = e_slot[eo]
    spos = spos[eo]

    grp = e_core * nw + e_w
    cnt = np.bincount(grp, minlength=n_cores * nw).reshape(n_cores, nw)
    nb = int((cnt.max() + P - 1) // P)
    cnt_w = cnt.max(axis=0)              # max edges per window across cores
    tail_w = cnt_w - (np.maximum(cnt_w, 1) - 1) // P * P  # last-batch lanes

    start = np.zeros(n_cores * nw + 1, np.int64)
    start[1:] = np.cumsum(cnt.reshape(-1))
    e_k = np.arange(grp.shape[0]) - start[grp]
    e_lane = e_k % P
    e_batch = e_k // P

    src_pack = np.zeros((n_cores, P, nw * nb), np.int32)
    slot_pack = np.full((n_cores, P, nw * nb), PAD_SLOT, np.float32)
    col = e_w * nb + e_batch
    src_pack[e_core, e_lane, col] = spos.astype(np.int32)
    slot_pack[e_core, e_lane, col] = e_slot

    tail_w = np.maximum(tail_w, 2)       # single-lane indirect DMA unsupported

    deg_pack = np.ones((n_cores, P, nw), np.float32)
    gp = pos_of[node_order]
    deg_pack[gp // ns, (gp % ns) % P, (gp % ns) // P] = deg[node_order]

    x_sh = np.ascontiguousarray(
        x[node_order].reshape(n_cores, ns, x.shape[1])
    ).astype(np.float32)

    nb_w = ((cnt.max(axis=0) + P - 1) // P).astype(np.int64)
    cfg = dict(n=n, ns=ns, nw=nw, nb=nb, nb_w=nb_w, tail_w=tail_w,
               n_cores=n_cores)
    per_core = dict(src_pack=src_pack, slot_pack=slot_pack, deg_pack=deg_pack,
                    x_sh=x_sh)
    return cfg, per_core, node_order


# ---------------------------------------------------------------- device side


def build(cfg, dims, debug=False, reps=1, pipe=True, zero_bias=False,
          nb_cap=None):
    n, ns, nw, nb, n_cores = cfg["n"], cfg["ns"], cfg["nw"], cfg["nb"], cfg["n_cores"]
    nb_w = cfg.get("nb_w", np.full(nw, nb))
    tail_w = cfg.get("tail_w", np.full(nw, P))
    fin0 = dims[0][0]
    fo4 = dims[3][1]
    rg = [list(range(n_cores))]

    nc = bacc.Bacc(None, target_bir_lowering=False)

    # ---- I/O
    x_in = nc.dram_tensor("x_sh", [ns, fin0], F32, kind="ExternalInput")
    srcp_in = nc.dram_tensor("src_pack", [P, nw * nb], I32, kind="ExternalInput")
    slotp_in = nc.dram_tensor("slot_pack", [P, nw * nb], BF16,
                              kind="ExternalInput")
    deg_in = nc.dram_tensor("deg_pack", [P, nw], F32, kind="ExternalInput")
    w_in, bt_in = [], []
    for li, (fi, fo) in enumerate(dims):
        w_in.append(nc.dram_tensor(f"W{li + 1}", [fi, fo], BF16,
                                   kind="ExternalInput"))
        bt_in.append(nc.dram_tensor(f"bt{li + 1}", [P, fo], F32,
                                    kind="ExternalInput"))
    iota_in = nc.dram_tensor("iota", [P, P], BF16, kind="ExternalInput")
    ident_in = nc.dram_tensor("ident", [P, P], BF16, kind="ExternalInput")
    out_t = nc.dram_tensor("out", [ns, fo4], F32, kind="ExternalOutput")

    # ---- internal DRAM: per-layer gather tables (full, via AllGather)
    hs_loc = [nc.dram_tensor(f"hsl{li}", [ns, dims[li][0]], BF16)
              for li in range(4)]
    hs_full = [
        nc.dram_tensor(f"hsf{li}", [n, dims[li][0]], BF16, addr_space="Shared")
        for li in range(4)
    ]

    last_ws = ns - (nw - 1) * P

    with tile.TileContext(nc) as tc:
        with (
            tc.tile_pool(name="const", bufs=1) as cpool,
            tc.tile_pool(name="gpool", bufs=4) as gpool,
            tc.tile_pool(name="hpool", bufs=4) as hpool,
            tc.tile_pool(name="spool", bufs=4) as spool,
            tc.tile_pool(name="acc", bufs=4, space="PSUM") as accp,
            tc.tile_pool(name="acc2", bufs=4, space="PSUM") as accp2,
        ):
            # ---------- constants
            iota_t = cpool.tile([P, P], BF16, tag="iota")
            nc.sync.dma_start(iota_t[:], iota_in[:, :])
            ident_t = cpool.tile([P, P], BF16, tag="ident")
            nc.sync.dma_start(ident_t[:], ident_in[:, :])
            w_t, bt_t = [], []
            for li, (fi, fo) in enumerate(dims):
                wt = cpool.tile([fi, fo], BF16, tag=f"w{li}")
                nc.sync.dma_start(wt[:], w_in[li][:, :])
                w_t.append(wt)
                bt = cpool.tile([P, fo], F32, tag=f"bt{li}")
                nc.sync.dma_start(bt[:], bt_in[li][:, :])
                bt_t.append(bt)
            src_t = cpool.tile([P, nw * nb], I32, tag="srcp")
            nc.sync.dma_start(src_t[:], srcp_in[:, :])
            slot_t = cpool.tile([P, nw * nb], BF16, tag="slotp")
            nc.sync.dma_start(slot_t[:], slotp_in[:, :])

            # ---------- dis = 1/sqrt(deg)
            deg_t = cpool.tile([P, nw], F32, tag="deg")
            nc.sync.dma_start(deg_t[:], deg_in[:, :])
            rec_t = cpool.tile([P, nw], F32, tag="rec")
            nc.vector.reciprocal(out=rec_t[:], in_=deg_t[:])
            dis_t = cpool.tile([P, nw], F32, tag="dis")
            nc.scalar.sqrt(out=dis_t[:], in_=rec_t[:])
            dis2_t = cpool.tile([P, nw], F32, tag="dis2")
            nc.vector.tensor_tensor(
                out=dis2_t[:], in0=dis_t[:], in1=dis_t[:],
                op=mybir.AluOpType.mult,
            )

            # ---------- persistent per-layer feature tiles (node w*128+p at
            # [p, w*fi : (w+1)*fi]); layer li's windows read their own rows
            # from hs_lay[li] for the self-loop identity matmul.
            nwf = nw - 1                     # count of full 128-node windows
            hs_lay = [
                cpool.tile([P, nw * dims[li][0]], BF16, tag=f"hslay{li}",
                           name=f"hslay{li}")
                for li in range(4)
            ]
            out_lay = cpool.tile([P, nw * fo4], F32, tag="outlay")

            def store_hs(lay, fo, loc):
                """hs_lay -> node-major DRAM table in 2 DMAs."""
                nc.sync.dma_start(
                    loc[: nwf * P, :].rearrange("(w p) f -> p w f", p=P),
                    lay[:, : nwf * fo].rearrange("p (w f) -> p w f", w=nwf),
                )
                nc.sync.dma_start(
                    loc[nwf * P :, :], lay[:last_ws, nwf * fo :]
                )

            # ---------- phase 0: hs0 = dis * x  (bf16)
            xstage = cpool.tile([P, nw * fin0], F32, tag="xstage")
            nc.sync.dma_start(
                xstage[:, : nwf * fin0].rearrange("p (w f) -> p w f", w=nwf),
                x_in[: nwf * P, :].rearrange("(w p) f -> p w f", p=P),
            )
            nc.sync.dma_start(
                xstage[:last_ws, nwf * fin0 :], x_in[nwf * P :, :]
            )
            for w in range(nw):
                ws = P if w < nw - 1 else last_ws
                nc.scalar.activation(
                    out=hs_lay[0][:ws, w * fin0 : (w + 1) * fin0],
                    in_=xstage[:ws, w * fin0 : (w + 1) * fin0],
                    func=mybir.ActivationFunctionType.Copy,
                    scale=dis_t[:ws, w : w + 1],
                )
            store_hs(hs_lay[0], fin0, hs_loc[0])
            nc.gpsimd.collective_compute(
                "AllGather",
                mybir.AluOpType.bypass,
                replica_groups=rg,
                ins=[hs_loc[0].ap().opt()],
                outs=[hs_full[0].ap().opt()],
            )

            # ---------- layers
            for rep in range(reps):
              for li in range(4):
                  fi, fo = dims[li]

                  def epi(zT_sb, ws, t0, w, li=li, fi=fi, fo=fo):
                      o = accp2.tile([P, fo], F32, tag="o")
                      nc.tensor.matmul(
                          out=o[:ws],
                          lhsT=zT_sb[:, :ws],
                          rhs=w_t[li][:],
                          start=True,
                          stop=True,
                      )
                      if zero_bias and li < 3:
                          nc.scalar.activation(
                              out=hs_lay[li + 1][:ws, w * fo : (w + 1) * fo],
                              in_=o[:ws],
                              func=mybir.ActivationFunctionType.Relu,
                              scale=dis2_t[:ws, w : w + 1],
                          )
                          return
                      if zero_bias:
                          v = spool.tile([P, fo], F32, tag="v")
                          nc.scalar.activation(
                              out=v[:ws],
                              in_=o[:ws],
                              func=mybir.ActivationFunctionType.Copy,
                              scale=dis_t[:ws, w : w + 1],
                          )
                      else:
                          u = spool.tile([P, fo], F32, tag="u")
                          nc.scalar.activation(
                              out=u[:ws],
                              in_=o[:ws],
                              func=mybir.ActivationFunctionType.Copy,
                              scale=dis_t[:ws, w : w + 1],
                          )
                          v = spool.tile([P, fo], F32, tag="v")
                          nc.vector.tensor_tensor(
                              out=v[:ws], in0=u[:ws], in1=bt_t[li][:ws],
                              op=mybir.AluOpType.add,
                          )
                      if li < 3:
                          nc.scalar.activation(
                              out=hs_lay[li + 1][:ws, w * fo : (w + 1) * fo],
                              in_=v[:ws],
                              func=mybir.ActivationFunctionType.Relu,
                              scale=dis_t[:ws, w : w + 1],
                          )
                      else:
                          nm = spool.tile([P, 1], F32, tag="nm")
                          nc.vector.tensor_reduce(
                              out=nm[:ws],
                              in_=v[:ws],
                              op=mybir.AluOpType.max,
                              axis=mybir.AxisListType.X,
                              negate=True,
                          )
                          e = spool.tile([P, fo], F32, tag="e")
                          nc.scalar.activation(
                              out=e[:ws],
                              in_=v[:ws],
                              func=mybir.ActivationFunctionType.Exp,
                              bias=nm[:ws],
                          )
                          s = spool.tile([P, 1], F32, tag="s")
                          nc.vector.tensor_reduce(
                              out=s[:ws],
                              in_=e[:ws],
                              op=mybir.AluOpType.add,
                              axis=mybir.AxisListType.X,
                          )
                          ls = spool.tile([P, 1], F32, tag="ls")
                          nc.scalar.activation(
                              out=ls[:ws], in_=s[:ws],
                              func=mybir.ActivationFunctionType.Ln,
                          )
                          nc.vector.tensor_scalar(
                              out=out_lay[:ws, w * fo : (w + 1) * fo],
                              in0=v[:ws],
                              scalar1=nm[:ws],
                              scalar2=ls[:ws],
                              op0=mybir.AluOpType.add,
                              op1=mybir.AluOpType.subtract,
                          )

                  pend = None
                  for wp in range(0, nw, 2):
                      pair = list(range(wp, min(wp + 2, nw)))
                      # one indirect DMA fetches both windows' edge rows
                      g = gpool.tile([P, 2 * nb * fi], BF16, tag="g")
                      nc.gpsimd.indirect_dma_start(
                          out=g[:, : len(pair) * nb * fi],
                          out_offset=None,
                          in_=hs_full[li][:, :],
                          in_offset=IndirectOffsetOnAxis(
                              ap=src_t[:, wp * nb : (wp + len(pair)) * nb],
                              axis=0,
                          ),
                      )
                      for j, w in enumerate(pair):
                          ws = P if w < nw - 1 else last_ws
                          t0 = w * P
                          nbw = int(nb_w[w]) if nb_cap is None else min(
                              int(nb_w[w]), nb_cap)
                          h = hpool.tile([P, nb * P], BF16, tag="h")
                          nc.vector.tensor_tensor(
                              out=h[:, : nbw * P].rearrange(
                                  "p (b s) -> p b s", b=nbw),
                              in0=slot_t[
                                  :, w * nb : w * nb + nbw
                              ].to_broadcast([P, nbw, P]),
                              in1=iota_t[:]
                              .rearrange("p (b s) -> p b s", b=1)
                              .to_broadcast([P, nbw, P]),
                              op=mybir.AluOpType.is_equal,
                          )
                          zT = accp.tile([fi, P], F32, tag="acc")
                          # self-loop contribution: zT[:, s] += hs[s] for
                          # the window's own nodes (kept resident in SBUF)
                          nc.tensor.matmul(
                              out=zT[:],
                              lhsT=hs_lay[li][:ws, w * fi : (w + 1) * fi],
                              rhs=ident_t[:ws],
                              start=True,
                              stop=False,
                          )
                          for b in range(nbw):
                              nc.tensor.matmul(
                                  out=zT[:],
                                  lhsT=g[:, (j * nb + b) * fi
                                         : (j * nb + b + 1) * fi],
                                  rhs=h[:, b * P : (b + 1) * P],
                                  start=False,
                                  stop=(b == nbw - 1),
                              )
                          zT_sb = spool.tile([fi, P], BF16, tag="zTsb")
                          nc.vector.tensor_copy(out=zT_sb[:], in_=zT[:])
                          if pipe:
                              if pend is not None:
                                  epi(*pend)
                              pend = (zT_sb, ws, t0, w)
                          else:
                              epi(zT_sb, ws, t0, w)
                  if pend is not None:
                      epi(*pend)

                  if li < 3:
                      store_hs(hs_lay[li + 1], dims[li + 1][0],
                               hs_loc[li + 1])
                      nc.gpsimd.collective_compute(
                          "AllGather",
                          mybir.AluOpType.bypass,
                          replica_groups=rg,
                          ins=[hs_loc[li + 1].ap().opt()],
                          outs=[hs_full[li + 1].ap().opt()],
                      )
                  else:
                      nc.sync.dma_start(
                          out_t[: nwf * P, :].rearrange(
                              "(w p) f -> p w f", p=P),
                          out_lay[:, : nwf * fo4].rearrange(
                              "p (w f) -> p w f", w=nwf),
                      )
                      nc.sync.dma_start(
                          out_t[nwf * P :, :], out_lay[:last_ws, nwf * fo4 :]
                      )

    nc.finalize()
    return nc


def make_in_maps(cfg, per_core, Ws, bs):
    common = dict(iota=np.tile(np.arange(P, dtype=NPBF), (P, 1)),
                  ident=np.eye(P, dtype=NPBF))
    for li in range(4):
        common[f"W{li + 1}"] = np.asarray(Ws[li], np.float32).astype(NPBF)
        common[f"bt{li + 1}"] = np.tile(np.asarray(bs[li], np.float32), (P, 1))
    in_maps = []
    for c in range(cfg["n_cores"]):
        m = dict(common)
        m["x_sh"] = per_core["x_sh"][c]
        m["src_pack"] = per_core["src_pack"][c]
        m["slot_pack"] = per_core["slot_pack"][c].astype(NPBF)
        m["deg_pack"] = per_core["deg_pack"][c]
        in_maps.append(m)
    return in_maps


# ---------------------------------------------------------------- entry point


def kernel(x, edge_index, W1, b1, W2, b2, W3, b3, W4, b4):
    x = np.asarray(x, np.float32)
    n_cores = 8
    cfg, per_core, node_order = preprocess(x, np.asarray(edge_index), n_cores)
    dims = [
        (W1.shape[0], W1.shape[1]),
        (W2.shape[0], W2.shape[1]),
        (W3.shape[0], W3.shape[1]),
        (W4.shape[0], W4.shape[1]),
    ]
    zb = all(not np.any(np.asarray(b)) for b in (b1, b2, b3, b4))
    nc = build(cfg, dims, zero_bias=zb)
    in_maps = make_in_maps(cfg, per_core, (W1, W2, W3, W4), (b1, b2, b3, b4))
    res = run_bass_kernel_spmd(nc, in_maps, list(range(n_cores)))
    outs = np.concatenate([res.results[c]["out"] for c in range(n_cores)], axis=0)
    full = np.empty((cfg["n"], dims[3][1]), np.float32)
    full[node_order] = outs
    return full



# revision 30
# speedup vs baseline: 1.1643x; 1.1135x over previous
"""4-layer GCN (N=100k, E=3.2M) on 8 TRN2 NeuronCores — aggregate-then-transform.

Key algebra: out = dis * ((A+I)-aggregate of dis*h) @ W + b, because the dense
transform commutes with the (linear) scatter-add aggregation. So we aggregate
the RAW pre-scaled features (fi-wide: 6/32/64/128) and apply W per-window
after aggregation; the [fi,ws]x[fi,fo] matmul also restores node-major
orientation for free. No per-node transform loops, no y tables, no PE
transposes anywhere.

Per layer: per dst-window -> indirect-DMA gather of hs[src] rows (fi wide),
one-hot H on DVE, scatter via PE matmul accumulation into zT [fi,P] PSUM,
W-matmul to o [ws,fo], epilogue dis-scale + bias (+relu, pre-scale for next
layer), write hs_next slice; AllGather hs_next across cores between layers.
Layer 4 ends with log_softmax on [ws,2].
"""

import sys

if "/opt/trn_rl_repo" not in sys.path:
    sys.path.insert(0, "/opt/trn_rl_repo")

import ml_dtypes
import numpy as np

import concourse.mybir as mybir
import concourse.tile as tile
from concourse import bacc
from concourse.bass import IndirectOffsetOnAxis
from concourse.bass_utils import run_bass_kernel_spmd

F32 = mybir.dt.float32
BF16 = mybir.dt.bfloat16
I32 = mybir.dt.int32
NPBF = ml_dtypes.bfloat16
P = 128
PAD_SLOT = 512.0

# ---------------------------------------------------------------- host side


def _assign_windows(deg, n_cores, ns):
    """Assign nodes to (core, window, slot), balancing per-window edge load."""
    n = deg.shape[0]
    nw = (ns + P - 1) // P
    last_cap = ns - (nw - 1) * P
    n_win = n_cores * nw
    caps = np.full(n_win, P, np.int64)
    caps[nw - 1 :: nw] = last_cap

    order = np.argsort(-deg, kind="stable")
    slots_of = [[] for _ in range(n_win)]
    win_ids = np.arange(n_win)
    pos = 0
    rnd = 0
    while pos < n:
        take = win_ids if rnd % 2 == 0 else win_ids[::-1]
        for wgid in take:
            if pos >= n:
                break
            if len(slots_of[wgid]) < caps[wgid]:
                slots_of[wgid].append(order[pos])
                pos += 1
        rnd += 1
    node_order = np.concatenate([np.array(s, np.int64) for s in slots_of])
    pos_of = np.empty(n, np.int64)
    pos_of[node_order] = np.arange(n)
    return node_order, pos_of, nw


def preprocess(x, edge_index, n_cores=8):
    n = x.shape[0]
    assert n % n_cores == 0
    ns = n // n_cores

    src = np.asarray(edge_index[0], np.int64)
    dst = np.asarray(edge_index[1], np.int64)

    # deg includes the self-loop (reference semantics); the self-loop edge
    # itself is NOT packed — it is applied on-device via an identity matmul.
    deg = np.bincount(dst, minlength=n) + 1
    node_order, pos_of, nw = _assign_windows(deg, n_cores, ns)

    dpos = pos_of[dst]
    spos = pos_of[src]
    e_core = dpos // ns
    off = dpos % ns
    e_w = off // P
    e_slot = off % P

    # sort edges by (core, window, src position) for gather locality
    key = (e_core * nw + e_w) * (n + 1) + spos
    eo = np.argsort(key, kind="stable")
    e_core = e_core[eo]
    e_w = e_w[eo]
    e_slot = e_slot[eo]
    spos = spos[eo]

    grp = e_core * nw + e_w
    cnt = np.bincount(grp, minlength=n_cores * nw).reshape(n_cores, nw)
    nb = int((cnt.max() + P - 1) // P)
    cnt_w = cnt.max(axis=0)              # max edges per window across cores
    tail_w = cnt_w - (np.maximum(cnt_w, 1) - 1) // P * P  # last-batch lanes

    start = np.zeros(n_cores * nw + 1, np.int64)
    start[1:] = np.cumsum(cnt.reshape(-1))
    e_k = np.arange(grp.shape[0]) - start[grp]
    e_lane = e_k % P
    e_batch = e_k // P

    src_pack = np.zeros((n_cores, P, nw * nb), np.int32)
    slot_pack = np.full((n_cores, P, nw * nb), PAD_SLOT, np.float32)
    col = e_w * nb + e_batch
    src_pack[e_core, e_lane, col] = spos.astype(np.int32)
    slot_pack[e_core, e_lane, col] = e_slot

    tail_w = np.maximum(tail_w, 2)       # single-lane indirect DMA unsupported

    deg_pack = np.ones((n_cores, P, nw), np.float32)
    gp = pos_of[node_order]
    deg_pack[gp // ns, (gp % ns) % P, (gp % ns) // P] = deg[node_order]

    x_sh = np.ascontiguousarray(
        x[node_order].reshape(n_cores, ns, x.shape[1])
    ).astype(np.float32)

    nb_w = ((cnt.max(axis=0) + P - 1) // P).astype(np.int64)
    cfg = dict(n=n, ns=ns, nw=nw, nb=nb, nb_w=nb_w, tail_w=tail_w,
               n_cores=n_cores)
    per_core = dict(src_pack=src_pack, slot_pack=slot_pack, deg_pack=deg_pack,
                    x_sh=x_sh)
    return cfg, per_core, node_order


# ---------------------------------------------------------------- device side


def build(cfg, dims, debug=False, reps=1, pipe=True, zero_bias=False,
          nb_cap=None):
    n, ns, nw, nb, n_cores = cfg["n"], cfg["ns"], cfg["nw"], cfg["nb"], cfg["n_cores"]
    nb_w = cfg.get("nb_w", np.full(nw, nb))
    tail_w = cfg.get("tail_w", np.full(nw, P))
    fin0 = dims[0][0]
    fo4 = dims[3][1]
    rg = [list(range(n_cores))]

    nc = bacc.Bacc(None, target_bir_lowering=False)

    # ---- I/O
    x_in = nc.dram_tensor("x_sh", [ns, fin0], F32, kind="ExternalInput")
    srcp_in = nc.dram_tensor("src_pack", [P, nw * nb], I32, kind="ExternalInput")
    slotp_in = nc.dram_tensor("slot_pack", [P, nw * nb], BF16,
                              kind="ExternalInput")
    deg_in = nc.dram_tensor("deg_pack", [P, nw], F32, kind="ExternalInput")
    w_in, bt_in = [], []
    for li, (fi, fo) in enumerate(dims):
        w_in.append(nc.dram_tensor(f"W{li + 1}", [fi, fo], BF16,
                                   kind="ExternalInput"))
        bt_in.append(nc.dram_tensor(f"bt{li + 1}", [P, fo], F32,
                                    kind="ExternalInput"))
    iota_in = nc.dram_tensor("iota", [P, P], BF16, kind="ExternalInput")
    ident_in = nc.dram_tensor("ident", [P, P], BF16, kind="ExternalInput")
    out_t = nc.dram_tensor("out", [ns, fo4], F32, kind="ExternalOutput")

    # ---- internal DRAM: per-layer gather tables (full, via AllGather)
    hs_loc = [nc.dram_tensor(f"hsl{li}", [ns, dims[li][0]], BF16)
              for li in range(4)]
    hs_full = [
        nc.dram_tensor(f"hsf{li}", [n, dims[li][0]], BF16, addr_space="Shared")
        for li in range(4)
    ]

    last_ws = ns - (nw - 1) * P

    with tile.TileContext(nc) as tc:
        with (
            tc.tile_pool(name="const", bufs=1) as cpool,
            tc.tile_pool(name="gpool", bufs=2) as gpool,
            tc.tile_pool(name="hpool", bufs=2) as hpool,
            tc.tile_pool(name="spool", bufs=4) as spool,
            tc.tile_pool(name="acc", bufs=4, space="PSUM") as accp,
            tc.tile_pool(name="acc2", bufs=4, space="PSUM") as accp2,
        ):
            # ---------- constants
            iota_t = cpool.tile([P, P], BF16, tag="iota")
            nc.sync.dma_start(iota_t[:], iota_in[:, :])
            ident_t = cpool.tile([P, P], BF16, tag="ident")
            nc.sync.dma_start(ident_t[:], ident_in[:, :])
            w_t, bt_t = [], []
            for li, (fi, fo) in enumerate(dims):
                wt = cpool.tile([fi, fo], BF16, tag=f"w{li}")
                nc.sync.dma_start(wt[:], w_in[li][:, :])
                w_t.append(wt)
                bt = cpool.tile([P, fo], F32, tag=f"bt{li}")
                nc.sync.dma_start(bt[:], bt_in[li][:, :])
                bt_t.append(bt)
            src_t = cpool.tile([P, nw * nb], I32, tag="srcp")
            nc.sync.dma_start(src_t[:], srcp_in[:, :])
            slot_t = cpool.tile([P, nw * nb], BF16, tag="slotp")
            nc.sync.dma_start(slot_t[:], slotp_in[:, :])

            # ---------- dis = 1/sqrt(deg)
            deg_t = cpool.tile([P, nw], F32, tag="deg")
            nc.sync.dma_start(deg_t[:], deg_in[:, :])
            rec_t = cpool.tile([P, nw], F32, tag="rec")
            nc.vector.reciprocal(out=rec_t[:], in_=deg_t[:])
            dis_t = cpool.tile([P, nw], F32, tag="dis")
            nc.scalar.sqrt(out=dis_t[:], in_=rec_t[:])
            dis2_t = cpool.tile([P, nw], F32, tag="dis2")
            nc.vector.tensor_tensor(
                out=dis2_t[:], in0=dis_t[:], in1=dis_t[:],
                op=mybir.AluOpType.mult,
            )

            # ---------- persistent per-layer feature tiles (node w*128+p at
            # [p, w*fi : (w+1)*fi]); layer li's windows read their own rows
            # from hs_lay[li] for the self-loop identity matmul.
            nwf = nw - 1                     # count of full 128-node windows
            hs_lay = [
                cpool.tile([P, nw * dims[li][0]], BF16, tag=f"hslay{li}",
                           name=f"hslay{li}")
                for li in range(4)
            ]
            out_lay = cpool.tile([P, nw * fo4], F32, tag="outlay")

            def store_hs(lay, fo, loc):
                """hs_lay -> node-major DRAM table in 2 DMAs."""
                nc.sync.dma_start(
                    loc[: nwf * P, :].rearrange("(w p) f -> p w f", p=P),
                    lay[:, : nwf * fo].rearrange("p (w f) -> p w f", w=nwf),
                )
                nc.sync.dma_start(
                    loc[nwf * P :, :], lay[:last_ws, nwf * fo :]
                )

            # ---------- phase 0: hs0 = dis * x  (bf16)
            xstage = cpool.tile([P, nw * fin0], F32, tag="xstage")
            nc.sync.dma_start(
                xstage[:, : nwf * fin0].rearrange("p (w f) -> p w f", w=nwf),
                x_in[: nwf * P, :].rearrange("(w p) f -> p w f", p=P),
            )
            nc.sync.dma_start(
                xstage[:last_ws, nwf * fin0 :], x_in[nwf * P :, :]
            )
            for w in range(nw):
                ws = P if w < nw - 1 else last_ws
                nc.scalar.activation(
                    out=hs_lay[0][:ws, w * fin0 : (w + 1) * fin0],
                    in_=xstage[:ws, w * fin0 : (w + 1) * fin0],
                    func=mybir.ActivationFunctionType.Copy,
                    scale=dis_t[:ws, w : w + 1],
                )
            store_hs(hs_lay[0], fin0, hs_loc[0])
            nc.gpsimd.collective_compute(
                "AllGather",
                mybir.AluOpType.bypass,
                replica_groups=rg,
                ins=[hs_loc[0].ap().opt()],
                outs=[hs_full[0].ap().opt()],
            )

            # ---------- layers
            for rep in range(reps):
              for li in range(4):
                  fi, fo = dims[li]

                  def epi(zT_sb, ws, t0, w, li=li, fi=fi, fo=fo):
                      o = accp2.tile([P, fo], F32, tag="o")
                      nc.tensor.matmul(
                          out=o[:ws],
                          lhsT=zT_sb[:, :ws],
                          rhs=w_t[li][:],
                          start=True,
                          stop=True,
                      )
                      if zero_bias and li < 3:
                          nc.scalar.activation(
                              out=hs_lay[li + 1][:ws, w * fo : (w + 1) * fo],
                              in_=o[:ws],
                              func=mybir.ActivationFunctionType.Relu,
                              scale=dis2_t[:ws, w : w + 1],
                          )
                          return
                      if zero_bias:
                          v = spool.tile([P, fo], F32, tag="v")
                          nc.scalar.activation(
                              out=v[:ws],
                              in_=o[:ws],
                              func=mybir.ActivationFunctionType.Copy,
                              scale=dis_t[:ws, w : w + 1],
                          )
                      else:
                          u = spool.tile([P, fo], F32, tag="u")
                          nc.scalar.activation(
                              out=u[:ws],
                              in_=o[:ws],
                              func=mybir.ActivationFunctionType.Copy,
                              scale=dis_t[:ws, w : w + 1],
                          )
                          v = spool.tile([P, fo], F32, tag="v")
                          nc.vector.tensor_tensor(
                              out=v[:ws], in0=u[:ws], in1=bt_t[li][:ws],
                              op=mybir.AluOpType.add,
                          )
                      if li < 3:
                          nc.scalar.activation(
                              out=hs_lay[li + 1][:ws, w * fo : (w + 1) * fo],
                              in_=v[:ws],
                              func=mybir.ActivationFunctionType.Relu,
                              scale=dis_t[:ws, w : w + 1],
                          )
                      else:
                          nm = spool.tile([P, 1], F32, tag="nm")
                          nc.vector.tensor_reduce(
                              out=nm[:ws],
                              in_=v[:ws],
                              op=mybir.AluOpType.max,
                              axis=mybir.AxisListType.X,
                              negate=True,
                          )
                          e = spool.tile([P, fo], F32, tag="e")
                          nc.scalar.activation(
                              out=e[:ws],
                              in_=v[:ws],
                              func=mybir.ActivationFunctionType.Exp,
                              bias=nm[:ws],
                          )
                          s = spool.tile([P, 1], F32, tag="s")
                          nc.vector.tensor_reduce(
                              out=s[:ws],
                              in_=e[:ws],
                              op=mybir.AluOpType.add,
                              axis=mybir.AxisListType.X,
                          )
                          ls = spool.tile([P, 1], F32, tag="ls")
                          nc.scalar.activation(
                              out=ls[:ws], in_=s[:ws],
                              func=mybir.ActivationFunctionType.Ln,
                          )
                          nc.vector.tensor_scalar(
                              out=out_lay[:ws, w * fo : (w + 1) * fo],
                              in0=v[:ws],
                              scalar1=nm[:ws],
                              scalar2=ls[:ws],
                              op0=mybir.AluOpType.add,
                              op1=mybir.AluOpType.subtract,
                          )

                  pend = None
                  for wp in range(0, nw, 4):
                      pair = list(range(wp, min(wp + 4, nw)))
                      # one indirect DMA fetches the group's edge rows
                      g = gpool.tile([P, 4 * nb * fi], BF16, tag="g")
                      nc.gpsimd.indirect_dma_start(
                          out=g[:, : len(pair) * nb * fi],
                          out_offset=None,
                          in_=hs_full[li][:, :],
                          in_offset=IndirectOffsetOnAxis(
                              ap=src_t[:, wp * nb : (wp + len(pair)) * nb],
                              axis=0,
                          ),
                      )
                      # slot one-hots built per window PAIR (pad batches
                      # produce all-zero columns; only b < nbw are consumed)
                      h_of = {}
                      for hj in range(0, len(pair), 2):
                          nwin = min(2, len(pair) - hj)
                          nbh = nb * nwin
                          h2 = hpool.tile([P, 2 * nb * P], BF16, tag="h")
                          nc.vector.tensor_tensor(
                              out=h2[:, : nbh * P].rearrange(
                                  "p (b s) -> p b s", b=nbh),
                              in0=slot_t[
                                  :,
                                  pair[hj] * nb : pair[hj] * nb + nbh,
                              ].to_broadcast([P, nbh, P]),
                              in1=iota_t[:]
                              .rearrange("p (b s) -> p b s", b=1)
                              .to_broadcast([P, nbh, P]),
                              op=mybir.AluOpType.is_equal,
                          )
                          h_of[hj] = h2
                      for j, w in enumerate(pair):
                          ws = P if w < nw - 1 else last_ws
                          t0 = w * P
                          nbw = int(nb_w[w]) if nb_cap is None else min(
                              int(nb_w[w]), nb_cap)
                          h = h_of[j - j % 2]
                          hoff = (j % 2) * nb
                          zT = accp.tile([fi, P], F32, tag="acc")
                          # self-loop contribution: zT[:, s] += hs[s] for
                          # the window's own nodes (kept resident in SBUF)
                          nc.tensor.matmul(
                              out=zT[:],
                              lhsT=hs_lay[li][:ws, w * fi : (w + 1) * fi],
                              rhs=ident_t[:ws],
                              start=True,
                              stop=False,
                          )
                          for b in range(nbw):
                              nc.tensor.matmul(
                                  out=zT[:],
                                  lhsT=g[:, (j * nb + b) * fi
                                         : (j * nb + b + 1) * fi],
                                  rhs=h[:, (hoff + b) * P
                                        : (hoff + b + 1) * P],
                                  start=False,
                                  stop=(b == nbw - 1),
                              )
                          zT_sb = spool.tile([fi, P], BF16, tag="zTsb")
                          nc.vector.tensor_copy(out=zT_sb[:], in_=zT[:])
                          if pipe:
                              if pend is not None:
                                  epi(*pend)
                              pend = (zT_sb, ws, t0, w)
                          else:
                              epi(zT_sb, ws, t0, w)
                  if pend is not None:
                      epi(*pend)

                  if li < 3:
                      store_hs(hs_lay[li + 1], dims[li + 1][0],
                               hs_loc[li + 1])
                      nc.gpsimd.collective_compute(
                          "AllGather",
                          mybir.AluOpType.bypass,
                          replica_groups=rg,
                          ins=[hs_loc[li + 1].ap().opt()],
                          outs=[hs_full[li + 1].ap().opt()],
                      )
                  else:
                      nc.sync.dma_start(
                          out_t[: nwf * P, :].rearrange(
                              "(w p) f -> p w f", p=P),
                          out_lay[:, : nwf * fo4].rearrange(
                              "p (w f) -> p w f", w=nwf),
                      )
                      nc.sync.dma_start(
                          out_t[nwf * P :, :], out_lay[:last_ws, nwf * fo4 :]
                      )

    nc.finalize()
    return nc


def make_in_maps(cfg, per_core, Ws, bs):
    common = dict(iota=np.tile(np.arange(P, dtype=NPBF), (P, 1)),
                  ident=np.eye(P, dtype=NPBF))
    for li in range(4):
        common[f"W{li + 1}"] = np.asarray(Ws[li], np.float32).astype(NPBF)
        common[f"bt{li + 1}"] = np.tile(np.asarray(bs[li], np.float32), (P, 1))
    in_maps = []
    for c in range(cfg["n_cores"]):
        m = dict(common)
        m["x_sh"] = per_core["x_sh"][c]
        m["src_pack"] = per_core["src_pack"][c]
        m["slot_pack"] = per_core["slot_pack"][c].astype(NPBF)
        m["deg_pack"] = per_core["deg_pack"][c]
        in_maps.append(m)
    return in_maps


# ---------------------------------------------------------------- entry point


def kernel(x, edge_index, W1, b1, W2, b2, W3, b3, W4, b4):
    x = np.asarray(x, np.float32)
    n_cores = 8
    cfg, per_core, node_order = preprocess(x, np.asarray(edge_index), n_cores)
    dims = [
        (W1.shape[0], W1.shape[1]),
        (W2.shape[0], W2.shape[1]),
        (W3.shape[0], W3.shape[1]),
        (W4.shape[0], W4.shape[1]),
    ]
    zb = all(not np.any(np.asarray(b)) for b in (b1, b2, b3, b4))
    nc = build(cfg, dims, zero_bias=zb)
    in_maps = make_in_maps(cfg, per_core, (W1, W2, W3, W4), (b1, b2, b3, b4))
    res = run_bass_kernel_spmd(nc, in_maps, list(range(n_cores)))
    outs = np.concatenate([res.results[c]["out"] for c in range(n_cores)], axis=0)
    full = np.empty((cfg["n"], dims[3][1]), np.float32)
    full[node_order] = outs
    return full

